# revision 8
# baseline (speedup 1.0000x reference)
"""Trainium2 Bass kernel for nn_Actor_attf_single (gnn_message_passing).

Strategy: pure data-parallel over 8 NeuronCores (32768 rows each).
Host pre-transposes the input to feature-major [97, B_core] bf16 (row 96 = ones
for bias folding into the matmuls); the device computes everything in
feature-major layout (batch on the free dim) and writes a packed [NG, 128, 512]
fp32 output that the host unpacks/transposes back to [B, 2] fp32.

Softmax normalization is never computed explicitly: pool = wsum/Z cancels
through LayerNorm, leaving LN = wc * g * rsqrt(S2'/16 + eps*Z^2) + b where
wc = mean-centered unnormalized weighted sum (computed directly by a
"centering" combination matmul) and Z enters only through the eps term.
"""
import os
import sys

sys.path.insert(0, "/opt/trn_rl_repo")

import numpy as np
import ml_dtypes

import concourse.bass as bass
import concourse.bacc as bacc
import concourse.tile as tile
import concourse.mybir as mybir

BF16 = mybir.dt.bfloat16
F32 = mybir.dt.float32
bfdt = ml_dtypes.bfloat16

N_AGENTS = 16
K = N_AGENTS - 1
D = 16
H = 32
EPS = 1e-5
SEPS = float(np.sqrt(EPS))

B = 262144
N_CORES = 8
B_CORE = B // N_CORES     # 32768
T = 512                   # batch tile (free dim per op)
TG = 4                    # tiles per group (input DMA + output packing)


# ----------------------------------------------------------------------------
# Host-side weight construction (fp32; cast later)
# ----------------------------------------------------------------------------
def _build_weights(w):
    out = {}
    W1all = np.zeros((97, 1024), np.float32)
    for k in range(K):
        c = 32 * k
        W1all[4 + 2 * k, c:c + 32] = w["oa_W1"][0]
        W1all[5 + 2 * k, c:c + 32] = w["oa_W1"][1]
        W1all[34 + 2 * k, c:c + 32] = w["oa_W1"][2]
        W1all[35 + 2 * k, c:c + 32] = w["oa_W1"][3]
        W1all[96, c:c + 32] = w["oa_b1"]
    W1all[0:4, 480:512] = w["en_W1"]
    W1all[96, 480:512] = w["en_b1"]
    for g in range(N_AGENTS):
        c = 512 + 32 * g
        W1all[64 + 2 * g, c:c + 32] = w["goal_W1"][0]
        W1all[65 + 2 * g, c:c + 32] = w["goal_W1"][1]
        W1all[96, c:c + 32] = w["goal_b1"]
    out["W1all"] = W1all

    W2A = np.zeros((128, 256), np.float32)
    bE01 = np.zeros((2, 128), np.float32)
    for m in range(4):
        e, h = divmod(m, 2)
        for a in range(4):
            s = 4 * h + a
            k = 8 * e + s
            blk = slice(64 * m + 16 * a, 64 * m + 16 * a + 16)
            if k < K:
                W2A[32 * a:32 * a + 32, blk] = w["oa_W2"]
                bE01[e, 16 * s:16 * s + 16] = w["oa_b2"]
            elif e == 1 and s == 7:
                W2A[32 * a:32 * a + 32, blk] = w["en_W2"]
                bE01[e, 16 * s:16 * s + 16] = w["en_b2"]
    out["W2A"] = W2A
    out["bE0"] = bE01[0][:, None]
    out["bE1"] = bE01[1][:, None]

    W2B = np.zeros((128, 256), np.float32)
    for m in range(4):
        for a in range(4):
            W2B[32 * a:32 * a + 32, 64 * m + 16 * a:64 * m + 16 * a + 16] = w["goal_W2"]
    out["W2B"] = W2B
    out["bE23"] = np.tile(w["goal_b2"], 8)[:, None]

    out["W2S"] = np.tile(w["en_W2"], (1, 8))
    out["bE4"] = np.tile(w["en_b2"], 8)[:, None]

    SC = np.zeros((128, 512), np.float32)
    for X in range(4):
        for s in range(8):
            if (X < 2 and 8 * X + s < K) or X >= 2:
                SC[16 * s:16 * s + 16,
                   128 * X + 16 * s:128 * X + 16 * s + 16] = 1.0
    out["SC"] = SC

    C2 = np.zeros((128, 256), np.float32)
    cblk = (np.eye(16, dtype=np.float32)
            + np.full((16, 16), -1.0 / 16, np.float32))
    for X in range(4):
        base = 64 * X
        for s in range(8):
            rows = slice(16 * s, 16 * s + 16)
            if X >= 2:  # goal chunks -> cols 0-15 (merged order: food first)
                C2[rows, base + 0:base + 16] = cblk
                C2[rows, base + 32] = SEPS / 16
            elif 8 * X + s < K:  # other chunks -> cols 16-31
                C2[rows, base + 16:base + 32] = cblk
                C2[rows, base + 33] = SEPS / 16
    out["C2"] = C2

    SQ1 = np.zeros((34, 2), np.float32)
    SQ1[0:16, 0] = 1.0 / 16
    SQ1[32, 0] = 1.0
    SQ1[16:32, 1] = 1.0 / 16
    SQ1[33, 1] = 1.0
    out["SQ1b"] = SQ1[:, [0] * 16 + [1] * 16]  # [34, 32] broadcast version

    out["gvec"] = np.concatenate([w["goal_ln_g"], w["oa_ln_g"]])[:, None]
    out["betav"] = np.concatenate([w["goal_ln_b"], w["oa_ln_b"]])[:, None]

    out["A1self"] = np.ascontiguousarray(w["act_W1"][0:16])
    out["A1rest"] = np.ascontiguousarray(w["act_W1"][16:48])
    out["b1act"] = w["act_b1"][:, None]
    out["b1act99"] = 99.0 * w["act_b1"][:, None]
    out["A2"] = 0.01 * w["act_W2"]
    out["b2act"] = w["act_b2"][:, None]
    out["b2act99"] = 99.0 * w["act_b2"][:, None]
    A3pad = np.zeros((32, 32), np.float32)
    A3pad[:, 0:2] = 0.01 * w["act_W3"]
    out["A3pad"] = A3pad
    b3act32 = np.zeros((32, 1), np.float32)
    b3act32[0] = w["act_b3"][0]
    b3act32[1] = w["act_b3"][1]
    out["b3act32"] = b3act32
    return out


BF16_NAMES = ["W1all", "W2A", "W2B", "W2S", "SC", "C2", "SQ1b",
              "A1self", "A1rest", "A2", "A3pad"]
F32_NAMES = ["bE0", "bE1", "bE23", "bE4", "gvec", "betav",
             "b1act", "b2act", "b3act32", "b1act99", "b2act99"]


# ----------------------------------------------------------------------------
# Device kernel body (Tile framework)
# ----------------------------------------------------------------------------
def build_kernel_body(tc, aps, b_core):
    nc = tc.nc
    Alu = mybir.AluOpType
    Act = mybir.ActivationFunctionType
    NT = b_core // T
    NG = NT // TG
    QUAKE = 0x5F3759DF + 1

    import contextlib
    ctx = contextlib.ExitStack()
    with ctx:
        wpool = ctx.enter_context(tc.tile_pool(name="wpool", bufs=1))
        inpool = ctx.enter_context(tc.tile_pool(name="inpool", bufs=2))
        hpool = ctx.enter_context(tc.tile_pool(name="hpool", bufs=2))
        epool = ctx.enter_context(tc.tile_pool(name="epool", bufs=2))
        ppool = ctx.enter_context(tc.tile_pool(name="ppool", bufs=2))
        spool = ctx.enter_context(tc.tile_pool(name="spool", bufs=2))
        opool = ctx.enter_context(tc.tile_pool(name="opool", bufs=2))

        # PSUM: 8 banks total, statically partitioned:
        #  PA "quad" [128,1024] x2 = 4 banks: l1a pair -> e01/e23 -> sb pair
        #  PB "duo"  [128,1024] x1 = 2 banks: l1b1 -> l1b2 -> e4
        #  PC "uno"  [128,512]  x2 = 2 banks: xw, y, h1p, h2p, h3p
        psA = ctx.enter_context(tc.tile_pool(name="psA", bufs=2, space="PSUM"))
        psB = ctx.enter_context(tc.tile_pool(name="psB", bufs=1, space="PSUM"))
        psC = ctx.enter_context(tc.tile_pool(name="psC", bufs=2, space="PSUM"))

        wsb = {}
        for nm in BF16_NAMES + F32_NAMES:
            ap = aps[nm]
            if nm == "W2S":
                t_ = wpool.tile([128, ap.shape[1]], ap.dtype, tag=nm)
                nc.sync.dma_start(out=t_[96:128, :], in_=ap[:])
                wsb[nm] = t_[96:128, :]
            else:
                t_ = wpool.tile(list(ap.shape), ap.dtype, tag=nm)
                nc.sync.dma_start(out=t_[:], in_=ap[:])
                wsb[nm] = t_

        for g in range(NG):
            xt = inpool.tile([97, TG * T], BF16, tag="xt")
            nc.sync.dma_start(out=xt[:], in_=aps["xt"][:, g * TG * T:(g + 1) * TG * T])
            og = opool.tile([128, T], F32, tag="og")

            for tau in range(TG):
                rhsx = xt[:, tau * T:(tau + 1) * T]

                # ---------- L1: other+self (l1a pair), goals (l1b 1/2) ------
                l1a1 = psA.tile([128, 1024], F32, tag="quad")
                l1a2 = psA.tile([128, 1024], F32, tag="quad")
                for b_ in range(2):
                    nc.tensor.matmul(l1a1[:, 512 * b_:512 * b_ + 512],
                                     wsb["W1all"][:, 128 * b_:128 * (b_ + 1)],
                                     rhsx, start=True, stop=True)
                    nc.tensor.matmul(l1a2[:, 512 * b_:512 * b_ + 512],
                                     wsb["W1all"][:, 256 + 128 * b_:256 + 128 * (b_ + 1)],
                                     rhsx, start=True, stop=True)
                ha = hpool.tile([128, 2048], BF16, tag="ha")
                nc.scalar.activation(ha[:, 0:1024], l1a1[:], Act.Relu)
                nc.scalar.activation(ha[:, 1024:2048], l1a2[:], Act.Relu)

                hb = hpool.tile([128, 2048], BF16, tag="hb")
                for half in range(2):
                    l1b = psB.tile([128, 1024], F32, tag="duo")
                    for b_ in range(2):
                        col = 512 + 256 * half + 128 * b_
                        nc.tensor.matmul(l1b[:, 512 * b_:512 * b_ + 512],
                                         wsb["W1all"][:, col:col + 128],
                                         rhsx, start=True, stop=True)
                    nc.vector.tensor_scalar(
                        out=hb[:, 1024 * half:1024 * half + 1024], in0=l1b[:],
                        scalar1=0.0, scalar2=None, op0=Alu.max)

                # ---------- L2 / enc ----------
                esb = []
                for pair in range(2):          # pair 0: E0|E1 (other), 1: E2|E3 (goal)
                    ep = psA.tile([128, 1024], F32, tag="quad")
                    W2 = wsb["W2A"] if pair == 0 else wsb["W2B"]
                    hsrc = ha if pair == 0 else hb
                    for e_ in range(2):
                        for half in range(2):
                            m = 2 * e_ + half
                            nc.tensor.matmul(
                                ep[64 * half:64 * half + 64, 512 * e_:512 * e_ + 512],
                                W2[:, 64 * m:64 * m + 64],
                                hsrc[:, 512 * m:512 * m + 512],
                                start=True, stop=True)
                    if pair == 0:
                        e0 = epool.tile([128, T], BF16, tag="e0")
                        nc.scalar.activation(e0[:], ep[:, 0:512], Act.Relu,
                                             bias=wsb["bE0"][:])
                        e1 = epool.tile([128, T], BF16, tag="e1")
                        nc.scalar.activation(e1[:], ep[:, 512:1024], Act.Relu,
                                             bias=wsb["bE1"][:])
                        esb += [e0, e1]
                    else:
                        e23 = epool.tile([128, 1024], BF16, tag="e23")
                        nc.vector.tensor_scalar(
                            out=e23[:], in0=ep[:], scalar1=wsb["bE23"][:],
                            scalar2=0.0, op0=Alu.add, op1=Alu.max)
                        esb += [e23[:, 0:512], e23[:, 512:1024]]

                e4p = psB.tile([128, 1024], F32, tag="duo")
                nc.tensor.matmul(e4p[:, 0:512], wsb["W2S"][:],
                                 ha[96:128, 1536:2048],
                                 start=True, stop=True, tile_position=(96, 0))
                selfb = epool.tile([128, T], BF16, tag="selfb")
                nc.scalar.activation(selfb[:], e4p[:, 0:512], Act.Relu,
                                     bias=wsb["bE4"][:])

                # ---------- scores (broadcast over d) + exp ----------
                sb1 = psA.tile([128, 1024], F32, tag="quad")
                sb2 = psA.tile([128, 1024], F32, tag="quad")
                eb = spool.tile([128, 2048], BF16, tag="eb")
                for X in range(4):
                    p1 = ppool.tile([128, T], BF16, tag=f"p1_{X}")
                    eng = nc.vector
                    eng.tensor_mul(p1[:], selfb[:], esb[X][:])
                    sbt = sb1 if X < 2 else sb2
                    nc.tensor.matmul(sbt[:, 512 * (X % 2):512 * (X % 2) + 512],
                                     wsb["SC"][:, 128 * X:128 * (X + 1)],
                                     p1[:], start=True, stop=True)
                nc.scalar.activation(eb[:, 0:1024], sb1[:], Act.Exp, scale=0.25)
                nc.scalar.activation(eb[:, 1024:2048], sb2[:], Act.Exp, scale=0.25)

                # ---------- pool / comb2 ----------
                xw = psC.tile([64, T], F32, tag="uno")
                for X in range(4):
                    p2 = ppool.tile([128, T], BF16, tag=f"p2_{X}")
                    eng = nc.vector
                    eng.tensor_mul(p2[:], eb[:, 512 * X:512 * X + 512], esb[X][:])
                    nc.tensor.matmul(xw[0:34, :],
                                     wsb["C2"][:, 64 * X:64 * X + 34],
                                     p2[:], start=(X == 0), stop=(X == 3))

                sq = spool.tile([34, T], BF16, tag="sq")
                nc.scalar.activation(sq[:], xw[0:34, :], Act.Square)
                y = psC.tile([32, T], F32, tag="uno")
                nc.tensor.matmul(y[:], wsb["SQ1b"][:], sq[:], start=True, stop=True)

                # ---------- rsqrt via quake seed + 1 Newton (no ACT table) --
                yi = y[:].bitcast(mybir.dt.int32)
                u1 = spool.tile([32, T], mybir.dt.int32, tag="u1")
                nc.vector.tensor_scalar(out=u1[:], in0=yi, scalar1=-1,
                                        scalar2=1, op0=Alu.bitwise_xor,
                                        op1=Alu.arith_shift_right)
                y0i = spool.tile([32, T], mybir.dt.int32, tag="y0i")
                nc.vector.tensor_scalar(out=y0i[:], in0=u1[:], scalar1=QUAKE,
                                        scalar2=None, op0=Alu.add)
                y0 = y0i[:].bitcast(F32)
                t3 = spool.tile([32, T], F32, tag="t3")
                nc.scalar.activation(t3[:], y0, Act.Square)
                t4 = spool.tile([32, T], F32, tag="t4")
                nc.vector.tensor_mul(t4[:], t3[:], y[:])
                t5 = spool.tile([32, T], F32, tag="t5")
                nc.vector.tensor_scalar(out=t5[:], in0=t4[:], scalar1=-0.5,
                                        scalar2=1.5, op0=Alu.mult, op1=Alu.add)
                ab = spool.tile([32, T], F32, tag="ab")
                nc.vector.tensor_mul(ab[:], t5[:], y0)

                # ---------- LN apply ----------
                lnt = spool.tile([32, T], BF16, tag="lnt")
                nc.vector.scalar_tensor_tensor(
                    out=lnt[:], in0=xw[0:32, :], scalar=wsb["gvec"][:],
                    in1=ab[:], op0=Alu.mult, op1=Alu.mult)
                merged = spool.tile([32, T], BF16, tag="merged")
                nc.vector.tensor_scalar(out=merged[:], in0=lnt[:],
                                        scalar1=wsb["betav"][:], scalar2=0.0,
                                        op0=Alu.add, op1=Alu.max)

                # ---------- act MLP ----------
                h1p = psC.tile([32, T], F32, tag="uno")
                nc.tensor.matmul(h1p[:], wsb["A1self"][:], selfb[0:16, :],
                                 start=True, stop=False)
                nc.tensor.matmul(h1p[:], wsb["A1rest"][:], merged[:],
                                 start=False, stop=True)
                r99 = spool.tile([32, T], BF16, tag="r99a")
                nc.scalar.activation(r99[:], h1p[:], Act.Relu,
                                     bias=wsb["b1act99"][:], scale=99.0)
                h1 = spool.tile([32, T], BF16, tag="h1sb")
                nc.vector.scalar_tensor_tensor(
                    out=h1[:], in0=h1p[:], scalar=wsb["b1act"][:],
                    in1=r99[:], op0=Alu.add, op1=Alu.add)

                h2p = psC.tile([32, T], F32, tag="uno")
                nc.tensor.matmul(h2p[:], wsb["A2"][:], h1[:], start=True, stop=True)
                r99b = spool.tile([32, T], BF16, tag="r99b")
                nc.scalar.activation(r99b[:], h2p[:], Act.Relu,
                                     bias=wsb["b2act99"][:], scale=99.0)
                h2 = spool.tile([32, T], BF16, tag="h2sb")
                nc.vector.scalar_tensor_tensor(
                    out=h2[:], in0=h2p[:], scalar=wsb["b2act"][:],
                    in1=r99b[:], op0=Alu.add, op1=Alu.add)

                h3p = psC.tile([32, T], F32, tag="uno")
                nc.tensor.matmul(h3p[:], wsb["A3pad"][:], h2[:], start=True, stop=True)
                nc.scalar.activation(og[32 * tau:32 * tau + 32, :], h3p[:],
                                     Act.Tanh, bias=wsb["b3act32"][:])

            nc.sync.dma_start(out=aps["out"][g, :, :], in_=og[:])


def build_nc(b_core, debug=False, num_devices=8):
    nc = bacc.Bacc("TRN2", target_bir_lowering=False, debug=debug,
                   num_devices=num_devices)
    NG = (b_core // T) // TG
    aps = {}
    aps["xt"] = nc.dram_tensor("xt", [97, b_core], BF16, kind="ExternalInput").ap()
    shapes = {
        "W1all": [97, 1024], "W2A": [128, 256], "W2B": [128, 256],
        "W2S": [32, 128], "SC": [128, 512], "C2": [128, 256],
        "SQ1b": [34, 32], "A1self": [16, 32], "A1rest": [32, 32],
        "A2": [32, 32], "A3pad": [32, 32],
        "bE0": [128, 1], "bE1": [128, 1], "bE23": [128, 1], "bE4": [128, 1],
        "gvec": [32, 1], "betav": [32, 1], "b1act": [32, 1], "b2act": [32, 1],
        "b3act32": [32, 1], "b1act99": [32, 1], "b2act99": [32, 1],
    }
    for nm in BF16_NAMES:
        aps[nm] = nc.dram_tensor(nm, shapes[nm], BF16, kind="ExternalInput").ap()
    for nm in F32_NAMES:
        aps[nm] = nc.dram_tensor(nm, shapes[nm], F32, kind="ExternalInput").ap()
    aps["out"] = nc.dram_tensor("out", [NG, 128, T], F32, kind="ExternalOutput").ap()

    with tile.TileContext(nc) as tc:
        build_kernel_body(tc, aps, b_core)
    nc.compile()
    return nc


def host_inputs(inputs, b_core=B_CORE, n_cores=N_CORES):
    """Build per-core in_maps from the harness inputs."""
    w32 = _build_weights({k: np.asarray(v, np.float32) for k, v in inputs.items()})
    wmap = {}
    for nm in BF16_NAMES:
        wmap[nm] = np.ascontiguousarray(w32[nm].astype(bfdt))
    for nm in F32_NAMES:
        wmap[nm] = np.ascontiguousarray(w32[nm])

    s = np.asarray(inputs["s_input"], np.float32)
    in_maps = []
    for c in range(n_cores):
        shard = s[c * b_core:(c + 1) * b_core]
        xt = np.empty((97, b_core), bfdt)
        xt[:96] = shard.astype(bfdt).T
        xt[96] = bfdt(1.0)
        in_maps.append({"xt": np.ascontiguousarray(xt), **wmap})
    return in_maps


def unpack_out(res_list, b_core=B_CORE):
    """res_list: per-core dict with 'out' [NG, 128, T] fp32 -> [B, 2] fp32."""
    outs = []
    for r in res_list:
        o = r["out"]                      # [NG, 128, T]
        NG = o.shape[0]
        o = o.reshape(NG, 4, 32, T)       # [g, tau, 32, T]
        oc = o[:, :, 0:2, :]              # [g, tau, c, T]
        oc = oc.transpose(0, 1, 3, 2)     # [g, tau, T, c]
        outs.append(oc.reshape(b_core, 2))
    return np.concatenate(outs, 0)


_NC_CACHE = {}


def kernel(**inputs):
    from concourse.bass_utils import run_bass_kernel_spmd
    key = "main"
    if key not in _NC_CACHE:
        _NC_CACHE[key] = build_nc(B_CORE, debug=False, num_devices=N_CORES)
    nc = _NC_CACHE[key]
    in_maps = host_inputs(inputs)
    trace = bool(int(os.environ.get("KERNEL_TRACE", "0")))
    res = run_bass_kernel_spmd(nc, in_maps, core_ids=list(range(N_CORES)),
                               trace=trace)
    out = unpack_out(res.results)
    kernel.last_exec_time_ns = res.exec_time_ns
    return out.astype(np.float32)
